# revision 1
# baseline (speedup 1.0000x reference)
"""Trainium2 Bass kernel for nn_CrossVariableMixingConv.

Reference computation (per row of x, B*L rows, C=862 channels):
    h   = conv1d(x, Wup, k=7, pad=3) + bup      # (RANK=8 channels)
    g   = gelu(h)  (erf-exact)
    d   = sum_r Wdown[r] * g[r] + bdown
    y   = LayerNorm(x + d) * gamma + beta       # LN over C

Sharding: pure data parallelism - the B*L = 11520 rows are split into 8
shards of 1440 rows, one per NeuronCore; the ~70 conv params are
replicated.

Per-core kernel structure (rows on partitions, chunks of 128 rows):
  - The k=7 conv is a bf16 matmul with the data stationary: lhsT = a
    host-pretransposed tap-slice [128 taps, rows] per window (8 windows
    of 108 output cols, partition-major in DRAM so each load is a clean
    2880B-per-line DMA), rhs = a banded weight matrix [128, 432]
    covering 4 ranks x 108 cols; a baked ones-row adds bup in-matmul.
  - Gelu (erf-exact LUT) runs on the Scalar engine out of PSUM writing
    fp8(e4m3) G in SBUF.  The Scalar engine is the critical path
    (1 elem/cycle/lane x 8 ranks x C), so everything else stays off it.
  - The rank contraction (Wdown) is 4 accumulating fp8 DoubleRow
    matmuls per half of C - each contracts TWO ranks per streamed
    column (out = sum_s W[:,s].T @ I[:,s]) against scaled fp8
    identities - halving the Tensor-engine column count.
  - The down-projection of chunk N is issued AFTER chunk N+1's conv
    matmuls (software pipelining), so the Scalar engine never waits for
    the PE queue to drain a down-projection.
  - Residual + LayerNorm run on the Vector engine in bf16
    (bn_stats/bn_aggr, mult-only Newton rstd, fused (y-mu)*rstd);
    results DMA out in bf16.

bdown is dropped: LayerNorm is invariant to a constant shift per row.
gamma/beta are applied only when not identity (ones/zeros here).
"""

import sys

for _p in ("/opt/trn_rl_repo",):
    if _p not in sys.path:
        sys.path.insert(0, _p)

import numpy as np
import ml_dtypes

B, L, C = 16, 720, 862
RANK, KTAPS = 8, 7
NCORES = 8
ROWS = B * L                 # 11520
RPC = ROWS // NCORES         # 1440 rows per core
PCH = 128                    # rows per chunk (partition dim)
NCHUNK = (RPC + PCH - 1) // PCH   # 12 (11 full + 1 of 32)
WW = 108                     # conv output columns per window
NW = 8                       # windows: 8*108 = 864 >= 862
CP = NW * WW                 # 864 padded output columns
CHALF = C // 2               # 431
EPS = 1e-5

_CACHE: dict = {}


def _build(apply_gamma_beta: bool):
    """Build + compile the per-core Bass program. Cached per flag."""
    key = ("nc", apply_gamma_beta)
    if key in _CACHE:
        return _CACHE[key]

    from contextlib import ExitStack

    import concourse.bacc as bacc
    import concourse.bass as bass
    import concourse.tile as tile
    from concourse import mybir

    f32 = mybir.dt.float32
    bf16 = mybir.dt.bfloat16
    fp8 = mybir.dt.float8e4
    AF = mybir.ActivationFunctionType
    ALU = mybir.AluOpType
    DR = mybir.MatmulPerfMode.DoubleRow

    nc = bacc.Bacc(
        "TRN2", target_bir_lowering=False, debug=False, num_devices=NCORES
    )

    xw_d = nc.dram_tensor("xw", [NW, 128, RPC], bf16, kind="ExternalInput").ap()
    xb_d = nc.dram_tensor("xb", [RPC, C], bf16, kind="ExternalInput").ap()
    band_d = nc.dram_tensor("band", [128, 2, 4 * WW], bf16, kind="ExternalInput").ap()
    wdi_d = nc.dram_tensor("wdi", [128, RANK, 128], bf16, kind="ExternalInput").ap()
    if apply_gamma_beta:
        gb_d = nc.dram_tensor("gb", [2, C], f32, kind="ExternalInput").ap()
    y_d = nc.dram_tensor("y", [RPC, C], bf16, kind="ExternalOutput").ap()

    with tile.TileContext(nc) as tc, ExitStack() as ctx:
        singles = ctx.enter_context(tc.tile_pool(name="singles", bufs=1))
        xp = ctx.enter_context(tc.tile_pool(name="xin", bufs=2))
        gp = ctx.enter_context(tc.tile_pool(name="g", bufs=3))
        op = ctx.enter_context(tc.tile_pool(name="o", bufs=3))
        stp = ctx.enter_context(tc.tile_pool(name="st", bufs=3))
        hp = ctx.enter_context(tc.tile_pool(name="hps", bufs=3, space="PSUM"))
        dp = ctx.enter_context(tc.tile_pool(name="dps", bufs=1, space="PSUM"))

        # Tiny weight tensors first so the first conv matmul waits only
        # on its own window slice, not the whole input stream.
        band_t = singles.tile([128, 2, 4 * WW], bf16)
        nc.sync.dma_start(out=band_t, in_=band_d)
        wdi_t = singles.tile([128, RANK, 128], bf16)
        nc.gpsimd.dma_start(out=wdi_t, in_=wdi_d)
        # Window tap-slices [128 taps, rows], partition-major loads
        # (2880B per line); partition 127 is the baked ones row.
        # All chunk-0 slices first, then the bulk, so the first chunk's
        # full window sweep is fed after ~0.5MB instead of 2.95MB.
        xtw = []
        for w in range(NW):
            xtw_t = singles.tile([128, RPC], bf16, tag=f"xtw{w}")
            xtw.append(xtw_t)
        for w in range(NW):
            eng = nc.sync if w % 2 == 0 else nc.gpsimd
            eng.dma_start(out=xtw[w][:, 0:PCH], in_=xw_d[w, :, 0:PCH])
        for w in range(NW):
            eng = nc.sync if w % 2 == 0 else nc.gpsimd
            eng.dma_start(out=xtw[w][:, PCH:RPC], in_=xw_d[w, :, PCH:RPC])
        if apply_gamma_beta:
            gamma_rep = singles.tile([128, C], f32)
            beta_rep = singles.tile([128, C], f32)
            for rep, row in ((gamma_rep, 0), (beta_rep, 1)):
                src = bass.AP(
                    tensor=gb_d.tensor,
                    offset=gb_d.offset + row * C,
                    ap=[[0, 128], [1, C]],
                )
                nc.gpsimd.dma_start(out=rep, in_=src)

        def emit_tail(G, xb_t, dT, n0, nr, split_halves=False):
            """Down-projection + residual/LN + store for a finished chunk.

            r-outer so each wdi_r stationary is loaded once (per-matmul
            LDWEIGHTS is the dominant PE tax).  The final chunk instead
            goes column-half-major (half 0 only needs windows 0-3's
            gelus), overlapping its down-projection + residual-add with
            the tail of the ACT sweep at the cost of 8 extra LDWs."""
            y_t = op.tile([128, C], bf16, tag="y")
            yc = y_t[:nr].rearrange("p (s c) -> p s c", s=2)
            xv = xb_t[:nr].rearrange("p (s c) -> p s c", s=2)
            if split_halves:
                for hh in range(2):
                    for r in range(RANK):
                        nc.tensor.matmul(
                            dT[:nr, hh, 0:CHALF],
                            lhsT=wdi_t[:nr, r, :nr],
                            rhs=G[:nr, r, hh * CHALF : (hh + 1) * CHALF],
                            start=(r == 0),
                            stop=(r == RANK - 1),
                        )
                    nc.vector.tensor_add(
                        out=yc[:, hh, :], in0=xv[:, hh, :], in1=dT[:nr, hh, 0:CHALF]
                    )
            else:
                for r in range(RANK):
                    for hh in range(2):
                        nc.tensor.matmul(
                            dT[:nr, hh, 0:CHALF],
                            lhsT=wdi_t[:nr, r, :nr],
                            rhs=G[:nr, r, hh * CHALF : (hh + 1) * CHALF],
                            start=(r == 0),
                            stop=(r == RANK - 1),
                        )
                nc.vector.tensor_add(out=yc, in0=xv, in1=dT[:nr, :, 0:CHALF])

            st = stp.tile([128, 2, 6], f32)
            for sg in range(2):
                nc.vector.bn_stats(out=st[:nr, sg, :], in_=yc[:, sg, :])
            mv = stp.tile([128, 2], f32, tag="mv")
            nc.vector.bn_aggr(out=mv[:nr], in_=st[:nr])

            # rstd = 1/sqrt(var+eps) on DVE only (keeps ACT pure-gelu):
            # u0 = 0.5 + 0.5/(var+eps), two mult-only Newton steps
            # u <- u*(1.5 - 0.5*(var+eps)*u^2).  var+eps ~ 1 here so the
            # seed is within ~1% and two steps are ample.
            v = stp.tile([128, 1], f32, tag="v")
            nc.vector.tensor_scalar_add(out=v[:nr], in0=mv[:nr, 1:2], scalar1=EPS)
            u = stp.tile([128, 1], f32, tag="u")
            nc.vector.reciprocal(out=u[:nr], in_=v[:nr])
            nc.vector.tensor_scalar(
                out=u[:nr], in0=u[:nr], scalar1=0.5, scalar2=0.5,
                op0=ALU.mult, op1=ALU.add,
            )
            t = stp.tile([128, 1], f32, tag="t")
            for _ in range(2):
                nc.vector.tensor_mul(t[:nr], u[:nr], u[:nr])
                nc.vector.tensor_mul(t[:nr], t[:nr], v[:nr])
                nc.vector.tensor_scalar(
                    out=t[:nr], in0=t[:nr], scalar1=-0.5, scalar2=1.5,
                    op0=ALU.mult, op1=ALU.add,
                )
                nc.vector.tensor_mul(u[:nr], u[:nr], t[:nr])

            o_t = op.tile([128, C], bf16, tag="o")
            nc.vector.tensor_scalar(
                out=o_t[:nr],
                in0=y_t[:nr],
                scalar1=mv[:nr, 0:1],
                scalar2=u[:nr],
                op0=ALU.subtract,
                op1=ALU.mult,
            )
            if apply_gamma_beta:
                nc.vector.tensor_mul(o_t[:nr], o_t[:nr], gamma_rep[:nr])
                nc.vector.tensor_add(o_t[:nr], o_t[:nr], beta_rep[:nr])
            nc.sync.dma_start(out=y_d[n0 : n0 + nr, :], in_=o_t[:nr])

        prev = None
        for ic in range(NCHUNK):
            n0 = ic * PCH
            nr = min(PCH, RPC - n0)

            xb_t = xp.tile([128, C], bf16)
            nc.gpsimd.dma_start(out=xb_t[:nr], in_=xb_d[n0 : n0 + nr, :])

            G = gp.tile([128, RANK, CP], bf16)
            # G[p, r, c] viewed as [p, rh, r4, w, i]: r = 4*rh + r4,
            # c = 108*w + i  (matches the conv matmul column order).
            Gv = G.rearrange("p (rh r4) (w i) -> p rh r4 w i", rh=2, w=NW)
            dT = dp.tile([128, 2, 512], f32)

            for w in range(NW):
                H = hp.tile([128, 2, 512], f32)
                for rh in range(2):
                    nc.tensor.matmul(
                        H[:nr, rh, 0 : 4 * WW],
                        lhsT=xtw[w][:, n0 : n0 + nr],
                        rhs=band_t[:, rh, :],
                        start=True,
                        stop=True,
                    )
                h_view = H[:nr, :, 0 : 4 * WW].rearrange(
                    "p rh (r4 i) -> p rh r4 i", i=WW
                )
                nc.scalar.activation(
                    out=Gv[:nr, :, :, w, :], in_=h_view, func=AF.Gelu
                )

            # Software pipeline: previous chunk's down-projection + LN are
            # issued after this chunk's convs so ACT is never queue-blocked.
            if prev is not None:
                emit_tail(*prev)
            prev = (G, xb_t, dT, n0, nr)

        emit_tail(*prev, split_halves=True)

    nc.compile()
    _CACHE[key] = nc
    return nc


def _host_prep(x, Wup, bup, Wdown, bdown, gamma, beta):
    """Build the per-core input maps (numpy only)."""
    bf = ml_dtypes.bfloat16
    f8 = ml_dtypes.float8_e4m3
    xf = np.ascontiguousarray(np.asarray(x, np.float32).reshape(ROWS, C))
    Wup_ = np.asarray(Wup, np.float32).reshape(RANK, KTAPS)
    bup_ = np.asarray(bup, np.float32).reshape(RANK)
    wd_ = np.asarray(Wdown, np.float32).reshape(RANK)
    gamma_ = np.asarray(gamma, np.float32).reshape(C)
    beta_ = np.asarray(beta, np.float32).reshape(C)

    # Transposed padded x [884, ROWS]: row p = xpad col p = x[:, p-3].
    xqt = np.zeros((884, ROWS), np.float32)
    xqt[3 : 3 + C, :] = xf.T
    # Window tap-slices, partition-major: xw[w, p, :] = xqt[108w + p],
    # with partition 127 = ones (bias row).
    xw = np.empty((NW, 128, ROWS), np.float32)
    for w in range(NW):
        xw[w, :127] = xqt[WW * w : WW * w + 127]
        xw[w, 127] = 1.0
    xw = xw.astype(bf)

    xb = xf.astype(bf)

    # Banded conv weights [tap, rh, r4*108 + i] (bf16):
    # band[i+k, rh, r4*WW+i] = Wup[4rh+r4, k]; band[127, rh, *] = bup.
    band = np.zeros((128, 2, 4 * WW), np.float32)
    i_idx = np.arange(WW)
    for r in range(RANK):
        rh, r4 = divmod(r, 4)
        for k in range(KTAPS):
            band[i_idx + k, rh, r4 * WW + i_idx] = Wup_[r, k]
        band[127, rh, r4 * WW : (r4 + 1) * WW] = bup_[r]
    band = band.astype(bf)

    # Scaled bf16 identities for the rank contraction.
    wdi = np.zeros((128, RANK, 128), np.float32)
    idx = np.arange(128)
    for r in range(RANK):
        wdi[idx, r, idx] = wd_[r]
    wdi = wdi.astype(bf)

    apply_gb = not (np.all(gamma_ == 1.0) and np.all(beta_ == 0.0))
    gb = np.stack([gamma_, beta_]).astype(np.float32)

    in_maps = []
    for i in range(NCORES):
        m = {
            "xw": np.ascontiguousarray(xw[:, :, i * RPC : (i + 1) * RPC]),
            "xb": xb[i * RPC : (i + 1) * RPC],
            "band": band,
            "wdi": wdi,
        }
        if apply_gb:
            m["gb"] = gb
        in_maps.append(m)
    return in_maps, apply_gb


def kernel(x, Wup, bup, Wdown, bdown, gamma, beta):
    from concourse.bass_utils import run_bass_kernel_spmd

    in_maps, apply_gb = _host_prep(x, Wup, bup, Wdown, bdown, gamma, beta)
    nc = _build(apply_gb)
    res = run_bass_kernel_spmd(nc, in_maps, core_ids=list(range(NCORES)))
    y = np.concatenate([res.results[i]["y"] for i in range(NCORES)], axis=0)
    return np.ascontiguousarray(
        y.astype(np.float32).reshape(B, L, C)
    )



# revision 3
# speedup vs baseline: 1.0001x; 1.0001x over previous
"""Trainium2 Bass kernel for nn_CrossVariableMixingConv.

Reference computation (per row of x, B*L rows, C=862 channels):
    h   = conv1d(x, Wup, k=7, pad=3) + bup      # (RANK=8 channels)
    g   = gelu(h)  (erf-exact)
    d   = sum_r Wdown[r] * g[r] + bdown
    y   = LayerNorm(x + d) * gamma + beta       # LN over C

Sharding: pure data parallelism - the B*L = 11520 rows are split into 8
shards of 1440 rows, one per NeuronCore; the ~70 conv params are
replicated.

Per-core structure (rows on partitions, 12 chunks of <=128 rows):
  - The Scalar engine (gelu at 1 elem/cycle/lane) is the hard floor:
    8 ranks x 862 cols x 1440 rows / 128 lanes / 1.2GHz ~= 69us.  The
    design minimizes ACT instruction count and keeps every other engine
    strictly below that budget.
  - Conv tiling: windows of 64 output cols; ONE matmul per window packs
    all 8 ranks (8x64 = 512 cols = exactly one PSUM bank).  lhsT is a
    host-pretransposed 72-tap x-slice (70 taps + ones row for bup), rhs
    is a single shared banded weight matrix [72, 512].  14 windows per
    chunk stream 7168 PE cycles.
  - PSUM: 6 banks = ring of 2 x [3-bank] buffers for conv output H;
    2 banks for the down-projection accumulator dT.  Gelu runs as 5
    ACTIVATEs per chunk (3+3+3+3+2 banks, 1536/1024 elems each),
    reading PSUM and writing bf16 G in SBUF.
  - Down-projection: 8 accumulating identity matmuls per column half
    (wd_r * G_r summed in PSUM), software-pipelined one chunk behind
    the conv so the Scalar engine never waits on the PE queue.
  - Residual + LayerNorm on the Vector engine (bn_stats/bn_aggr,
    mult-only Newton rstd, fused (y-mu)*rstd); bf16 DMA out.
  - Startup: a warmup gelu on a const tile pulls ACT_TABLE_LOAD to t=0;
    the band/first-chunk/second-chunk inputs are packed into two fused
    "head" DRAM tensors so the first matmul waits on one small DMA.

bdown is dropped: LayerNorm is invariant to a constant shift per row.
gamma/beta are applied only when not identity (ones/zeros here).
"""

import sys

for _p in ("/opt/trn_rl_repo",):
    if _p not in sys.path:
        sys.path.insert(0, _p)

import numpy as np
import ml_dtypes

B, L, C = 16, 720, 862
RANK, KTAPS = 8, 7
NCORES = 8
ROWS = B * L                 # 11520
RPC = ROWS // NCORES         # 1440 rows per core
PCH = 128                    # rows per chunk (partition dim)
NCHUNK = (RPC + PCH - 1) // PCH   # 12 (11 full + 1 of 32)
WW = 64                      # conv output columns per window
NW = 14                      # windows: 13*64 + 30 = 862 (rest padded)
CP = NW * WW                 # 896 padded output columns
TAPS = 72                    # 70 data taps + ones row (70) + zero row (71)
CHALF = 431
EPS = 1e-5
GROUPS = [(0, 3), (3, 3), (6, 3), (9, 3), (12, 2)]  # (first window, count)

# head1: band [72, 512] + chunk-0 window slices 14 x [72, 128]
H1W = 512 + NW * PCH         # 2304
# head2: chunk-1 window slices 14 x [72, 128] + wdi [128, 8*128]
H2W = NW * PCH + RANK * 128  # 2816
BULKW = RPC - 2 * PCH        # 1184

_CACHE: dict = {}


def _build(apply_gamma_beta: bool):
    """Build + compile the per-core Bass program. Cached per flag."""
    key = ("nc", apply_gamma_beta)
    if key in _CACHE:
        return _CACHE[key]

    from contextlib import ExitStack

    import concourse.bacc as bacc
    import concourse.bass as bass
    import concourse.tile as tile
    from concourse import mybir

    f32 = mybir.dt.float32
    bf16 = mybir.dt.bfloat16
    AF = mybir.ActivationFunctionType
    ALU = mybir.AluOpType

    nc = bacc.Bacc(
        "TRN2", target_bir_lowering=False, debug=False, num_devices=NCORES
    )

    h1_d = nc.dram_tensor("h1", [TAPS, H1W], bf16, kind="ExternalInput").ap()
    h2_d = nc.dram_tensor("h2", [128, H2W], bf16, kind="ExternalInput").ap()
    blk_d = nc.dram_tensor("blk", [NW, TAPS, BULKW], bf16, kind="ExternalInput").ap()
    xb_d = nc.dram_tensor("xb", [RPC, C], bf16, kind="ExternalInput").ap()
    if apply_gamma_beta:
        gb_d = nc.dram_tensor("gb", [2, C], f32, kind="ExternalInput").ap()
    y_d = nc.dram_tensor("y", [RPC, C], bf16, kind="ExternalOutput").ap()

    with tile.TileContext(nc) as tc, ExitStack() as ctx:
        singles = ctx.enter_context(tc.tile_pool(name="singles", bufs=1))
        xp = ctx.enter_context(tc.tile_pool(name="xin", bufs=2))
        gp = ctx.enter_context(tc.tile_pool(name="g", bufs=2))
        op = ctx.enter_context(tc.tile_pool(name="o", bufs=3))
        stp = ctx.enter_context(tc.tile_pool(name="st", bufs=3))
        hp = ctx.enter_context(tc.tile_pool(name="hps", bufs=2, space="PSUM"))
        dp = ctx.enter_context(tc.tile_pool(name="dps", bufs=1, space="PSUM"))

        # Warmup gelu on a const tile: pulls the ~1.3us ACT_TABLE_LOAD
        # to t~0 so it overlaps the input DMAs.
        warm = singles.tile([1, 2], f32)
        nc.vector.memset(warm, 0.0)
        nc.scalar.activation(out=warm, in_=warm, func=AF.Gelu)

        # Fused-head DMAs: one dispatch each so the first conv matmul
        # waits on a single ~0.6MB transfer, not a dispatch chain.
        h1_t = singles.tile([TAPS, H1W], bf16)
        nc.sync.dma_start(out=h1_t, in_=h1_d)
        h2_t = singles.tile([128, H2W], bf16)
        nc.gpsimd.dma_start(out=h2_t, in_=h2_d)
        band_t = h1_t[:, 0:512]
        wdi_t = h2_t[:, NW * PCH :].rearrange("p (r q) -> p r q", r=RANK)

        # Bulk window slices (rows 256:1440), needed from chunk 2 on.
        blk = []
        for w in range(NW):
            bt = singles.tile([TAPS, BULKW], bf16, tag=f"blk{w}")
            blk.append(bt)
        for w in range(NW):
            eng = nc.sync if w % 2 == 0 else nc.gpsimd
            eng.dma_start(out=blk[w], in_=blk_d[w])

        if apply_gamma_beta:
            gamma_rep = singles.tile([128, C], f32)
            beta_rep = singles.tile([128, C], f32)
            for rep, row in ((gamma_rep, 0), (beta_rep, 1)):
                src = bass.AP(
                    tensor=gb_d.tensor,
                    offset=gb_d.offset + row * C,
                    ap=[[0, 128], [1, C]],
                )
                nc.gpsimd.dma_start(out=rep, in_=src)

        def conv_lhs(w, ic, n0, nr):
            if ic == 0:
                return h1_t[:, 512 + PCH * w : 512 + PCH * w + nr]
            if ic == 1:
                return h2_t[0:TAPS, PCH * w : PCH * w + nr]
            return blk[w][:, n0 - 2 * PCH : n0 - 2 * PCH + nr]

        def emit_down(G, dTf, n0, nr, c0, cn):
            for r in range(RANK):
                nc.tensor.matmul(
                    dTf[:nr, c0 : c0 + cn],
                    lhsT=wdi_t[:nr, r, :nr],
                    rhs=G[:nr, r, c0 : c0 + cn],
                    start=(r == 0),
                    stop=(r == RANK - 1),
                )

        def emit_ln(y_t, n0, nr):
            """Stats + rstd + normalize + store for completed y_t."""
            yc = y_t[:nr].rearrange("p (s c) -> p s c", s=2)
            st = stp.tile([128, 2, 6], f32)
            for sg in range(2):
                nc.vector.bn_stats(out=st[:nr, sg, :], in_=yc[:, sg, :])
            mv = stp.tile([128, 2], f32, tag="mv")
            nc.vector.bn_aggr(out=mv[:nr], in_=st[:nr])

            # rstd = 1/sqrt(var+eps) on DVE only (keeps ACT pure-gelu):
            # u0 = 0.5 + 0.5/(var+eps), two mult-only Newton steps.
            v = stp.tile([128, 1], f32, tag="v")
            nc.vector.tensor_scalar_add(out=v[:nr], in0=mv[:nr, 1:2], scalar1=EPS)
            u = stp.tile([128, 1], f32, tag="u")
            nc.vector.reciprocal(out=u[:nr], in_=v[:nr])
            nc.vector.tensor_scalar(
                out=u[:nr], in0=u[:nr], scalar1=0.5, scalar2=0.5,
                op0=ALU.mult, op1=ALU.add,
            )
            t = stp.tile([128, 1], f32, tag="t")
            for _ in range(2):
                nc.vector.tensor_mul(t[:nr], u[:nr], u[:nr])
                nc.vector.tensor_mul(t[:nr], t[:nr], v[:nr])
                nc.vector.tensor_scalar(
                    out=t[:nr], in0=t[:nr], scalar1=-0.5, scalar2=1.5,
                    op0=ALU.mult, op1=ALU.add,
                )
                nc.vector.tensor_mul(u[:nr], u[:nr], t[:nr])

            o_t = op.tile([128, C], bf16, tag="o")
            nc.vector.tensor_scalar(
                out=o_t[:nr],
                in0=y_t[:nr],
                scalar1=mv[:nr, 0:1],
                scalar2=u[:nr],
                op0=ALU.subtract,
                op1=ALU.mult,
            )
            if apply_gamma_beta:
                nc.vector.tensor_mul(o_t[:nr], o_t[:nr], gamma_rep[:nr])
                nc.vector.tensor_add(o_t[:nr], o_t[:nr], beta_rep[:nr])
            nc.sync.dma_start(out=y_d[n0 : n0 + nr, :], in_=o_t[:nr])

        def emit_tail(G, xb_t, n0, nr):
            """Down-projection + residual/LN + store for a finished chunk."""
            dT = dp.tile([128, 2, 512], f32)
            dTf = dT.rearrange("p a b -> p (a b)")
            emit_down(G, dTf, n0, nr, 0, 512)
            emit_down(G, dTf, n0, nr, 512, C - 512)
            y_t = op.tile([128, C], bf16, tag="y")
            nc.vector.tensor_add(
                out=y_t[:nr], in0=xb_t[:nr], in1=dTf[:nr, 0:C]
            )
            emit_ln(y_t, n0, nr)

        prev = None
        for ic in range(NCHUNK):
            n0 = ic * PCH
            nr = min(PCH, RPC - n0)
            final = ic == NCHUNK - 1

            xb_t = xp.tile([128, C], bf16)
            nc.gpsimd.dma_start(out=xb_t[:nr], in_=xb_d[n0 : n0 + nr, :])

            G = gp.tile([128, RANK, CP], bf16)
            dTf = y_t = None

            for gi, (w0, cnt) in enumerate(GROUPS):
                H = hp.tile([128, 3, 512], f32)
                for j in range(cnt):
                    w = w0 + j
                    nc.tensor.matmul(
                        H[:nr, j, :],
                        lhsT=conv_lhs(w, ic, n0, nr),
                        rhs=band_t,
                        start=True,
                        stop=True,
                    )
                src = H[:nr, 0:cnt].rearrange("p s (r i) -> p s r i", i=WW)
                dst = G[:nr, :, WW * w0 : WW * (w0 + cnt)].rearrange(
                    "p r (w i) -> p w r i", i=WW
                )
                nc.scalar.activation(out=dst, in_=src, func=AF.Gelu)
                if final and gi == 2:
                    # Retire the previous chunk first (frees the single dp
                    # buffer), then overlap half-0 of the final tail with
                    # the remaining ACT groups (cols 0:512 <- windows 0-7).
                    if prev is not None:
                        emit_tail(*prev)
                        prev = None
                    dT = dp.tile([128, 2, 512], f32)
                    dTf = dT.rearrange("p a b -> p (a b)")
                    y_t = op.tile([128, C], bf16, tag="y")
                    emit_down(G, dTf, n0, nr, 0, 512)
                    nc.vector.tensor_add(
                        out=y_t[:nr, 0:512],
                        in0=xb_t[:nr, 0:512],
                        in1=dTf[:nr, 0:512],
                    )

            if not final:
                if prev is not None:
                    emit_tail(*prev)
                prev = (G, xb_t, n0, nr)

        # Final chunk: half 0 already merged above; finish half 1.
        emit_down(G, dTf, n0, nr, 512, C - 512)
        nc.vector.tensor_add(
            out=y_t[:nr, 512:C], in0=xb_t[:nr, 512:C], in1=dTf[:nr, 512:C]
        )
        emit_ln(y_t, n0, nr)

    nc.compile()
    _CACHE[key] = nc
    return nc


def _host_prep(x, Wup, bup, Wdown, bdown, gamma, beta):
    """Build the per-core input maps (numpy only)."""
    bf = ml_dtypes.bfloat16
    xf = np.ascontiguousarray(np.asarray(x, np.float32).reshape(ROWS, C))
    Wup_ = np.asarray(Wup, np.float32).reshape(RANK, KTAPS)
    bup_ = np.asarray(bup, np.float32).reshape(RANK)
    wd_ = np.asarray(Wdown, np.float32).reshape(RANK)
    gamma_ = np.asarray(gamma, np.float32).reshape(C)
    beta_ = np.asarray(beta, np.float32).reshape(C)

    # Transposed padded x [904, ROWS]: row m = x[:, m-3] (zero outside).
    xqt = np.zeros((904, ROWS), np.float32)
    xqt[3 : 3 + C, :] = xf.T
    # Window tap-slices [NW, 72, ROWS]: rows 0..69 = xqt[64w .. 64w+70],
    # row 70 = ones (bias), row 71 = zero pad.
    xw = np.zeros((NW, TAPS, ROWS), np.float32)
    for w in range(NW):
        xw[w, :70] = xqt[WW * w : WW * w + 70]
        xw[w, 70] = 1.0
    xw = xw.astype(bf)

    xb = xf.astype(bf)

    # Shared banded conv weights [72, 512] (bf16):
    # band[i+k, r*64+i] = Wup[r, k]; band[70, r*64+i] = bup[r].
    band = np.zeros((TAPS, 8 * WW), np.float32)
    i_idx = np.arange(WW)
    for r in range(RANK):
        for k in range(KTAPS):
            band[i_idx + k, r * WW + i_idx] = Wup_[r, k]
        band[70, r * WW : (r + 1) * WW] = bup_[r]
    band = band.astype(bf)

    # Scaled bf16 identities for the rank contraction.
    wdi = np.zeros((128, RANK, 128), np.float32)
    idx = np.arange(128)
    for r in range(RANK):
        wdi[idx, r, idx] = wd_[r]
    wdi = wdi.astype(bf)

    apply_gb = not (np.all(gamma_ == 1.0) and np.all(beta_ == 0.0))
    gb = np.stack([gamma_, beta_]).astype(np.float32)

    in_maps = []
    for i in range(NCORES):
        xwc = xw[:, :, i * RPC : (i + 1) * RPC]
        h1 = np.zeros((TAPS, H1W), bf)
        h1[:, 0:512] = band
        h2 = np.zeros((128, H2W), bf)
        for w in range(NW):
            h1[:, 512 + PCH * w : 512 + PCH * (w + 1)] = xwc[w, :, 0:PCH]
            h2[0:TAPS, PCH * w : PCH * (w + 1)] = xwc[w, :, PCH : 2 * PCH]
        h2[:, NW * PCH :] = wdi.reshape(128, RANK * 128)
        m = {
            "h1": h1,
            "h2": h2,
            "blk": np.ascontiguousarray(xwc[:, :, 2 * PCH :]),
            "xb": xb[i * RPC : (i + 1) * RPC],
        }
        if apply_gb:
            m["gb"] = gb
        in_maps.append(m)
    return in_maps, apply_gb


def kernel(x, Wup, bup, Wdown, bdown, gamma, beta):
    from concourse.bass_utils import run_bass_kernel_spmd

    in_maps, apply_gb = _host_prep(x, Wup, bup, Wdown, bdown, gamma, beta)
    nc = _build(apply_gb)
    res = run_bass_kernel_spmd(nc, in_maps, core_ids=list(range(NCORES)))
    y = np.concatenate([res.results[i]["y"] for i in range(NCORES)], axis=0)
    return np.ascontiguousarray(
        y.astype(np.float32).reshape(B, L, C)
    )
